# revision 26
# baseline (speedup 1.0000x reference)
"""Trainium2 Bass kernel for a dense transformer block (B=128,T=256,C=384,H=6).

Strategy: data-parallel over batch across 8 NeuronCores (16 batch elements
per core, processed as 8 pairs with a 512-wide fused token axis).  The whole
block is computed feature-major (channels on partitions) so no on-device
transposes are needed:
  - LN stats via ones-vector matmuls (partition-axis reduction on PE),
    broadcast back via a K=1 matmul.
  - QKV/proj/MLP as weight-stationary matmuls (weights pre-packed + LN gains
    and the C**-0.5 score scale folded in on host).
  - Scores computed transposed (key-major) so the reference's softmax over
    the *query* axis becomes a free-axis row softmax; causal masking via a
    triangular multiplicative mask fused into the row-sum (one DVE op); the
    fully-masked quadrant is never computed.
  - matmul operands bf16 (full PE rate), accumulation + residual path fp32.
"""

import os
import numpy as np
import ml_dtypes

import concourse.bacc as bacc
import concourse.bass as bass
import concourse.tile as tile
from concourse import mybir
from concourse.bass_utils import run_bass_kernel_spmd

F32 = mybir.dt.float32
BF16 = mybir.dt.bfloat16
AF = mybir.ActivationFunctionType
OP = mybir.AluOpType

B, T, C, H, HS = 128, 256, 384, 6, 64
NCORES = 8
BPC = B // NCORES          # batch elements per core
NPAIR = BPC // 2           # pairs per core
TT = 2 * T                 # fused pair token axis (512)
KC = C // 128              # 3 c-chunks
MU = 4 * C // 128          # 12 u-chunks
EPS = 1e-5

_CACHE = {}


def _build(npair=NPAIR, num_devices=NCORES, stage=99):
    nc = bacc.Bacc("TRN2", target_bir_lowering=False, debug=False,
                   num_devices=num_devices, enable_asserts=False)

    xf_d = nc.dram_tensor("xf", [npair, C, TT], F32, kind="ExternalInput").ap()
    xb_d = nc.dram_tensor("xb", [npair, C, TT], BF16, kind="ExternalInput").ap()
    wq_d = nc.dram_tensor("wq", [128, KC * C], BF16, kind="ExternalInput").ap()
    wk_d = nc.dram_tensor("wk", [128, KC * C], BF16, kind="ExternalInput").ap()
    wv_d = nc.dram_tensor("wv", [128, KC * C], BF16, kind="ExternalInput").ap()
    wp_d = nc.dram_tensor("wp", [128, KC * C], BF16, kind="ExternalInput").ap()
    w1_d = nc.dram_tensor("w1", [128, KC * 4 * C], BF16, kind="ExternalInput").ap()
    w2_d = nc.dram_tensor("w2", [128, MU * C], BF16, kind="ExternalInput").ap()
    bias_d = nc.dram_tensor("biases", [128, 24], F32, kind="ExternalInput").ap()
    bv_d = nc.dram_tensor("bv", [C], F32, kind="ExternalInput").ap()
    mask_d = nc.dram_tensor("trimask", [128, 128], BF16, kind="ExternalInput").ap()
    out_d = nc.dram_tensor("out", [npair, C, TT], F32, kind="ExternalOutput").ap()

    with tile.TileContext(nc) as tc:
        with (
            tc.tile_pool(name="consts", bufs=1) as cp,
            tc.tile_pool(name="p2", bufs=2) as p2,
            tc.tile_pool(name="p3", bufs=3) as p3,
            tc.tile_pool(name="pst", bufs=2) as pst,
            tc.tile_pool(name="pu", bufs=1) as pu,
            tc.tile_pool(name="pe3", bufs=6) as pe3,
            tc.tile_pool(name="ps", bufs=8, space="PSUM") as ps_p,
        ):
            ps_stat_p = ps_bc_p = ps_w_p = ps_s_p = ps_a_p = ps_p
            # ---- constants ----
            wq_sb = cp.tile([128, KC * C], BF16)
            nc.sync.dma_start(out=wq_sb, in_=wq_d)
            wk_sb = cp.tile([128, KC * C], BF16)
            nc.sync.dma_start(out=wk_sb, in_=wk_d)
            wv_sb = cp.tile([128, KC * C], BF16)
            nc.sync.dma_start(out=wv_sb, in_=wv_d)
            wp_sb = cp.tile([128, KC * C], BF16)
            nc.sync.dma_start(out=wp_sb, in_=wp_d)
            w1_sb = cp.tile([128, KC * 4 * C], BF16)
            nc.sync.dma_start(out=w1_sb, in_=w1_d)
            w2_sb = cp.tile([128, MU * C], BF16)
            nc.sync.dma_start(out=w2_sb, in_=w2_d)
            bias_sb = cp.tile([128, 24], F32)
            nc.sync.dma_start(out=bias_sb, in_=bias_d)
            bv_sb = cp.tile([128, C], F32)
            bv_bcast = bass.AP(tensor=bv_d.tensor, offset=bv_d.offset,
                               ap=[[0, 128]] + list(bv_d.ap))
            nc.sync.dma_start(out=bv_sb, in_=bv_bcast)
            mask_sb = cp.tile([128, 128], BF16)
            nc.sync.dma_start(out=mask_sb, in_=mask_d)
            ones_k = cp.tile([128, 1], BF16)
            nc.vector.memset(ones_k, 1.0)
            ones_b = cp.tile([1, 128], BF16)
            nc.vector.memset(ones_b, 1.0)
            eps_sb = cp.tile([1, 1], F32)
            nc.vector.memset(eps_sb, EPS)

            def layernorm(xin_b, xin_f, zout, tagp):
                """Feature-major LN: xin_b bf16 [128,KC,TT], writes zout bf16."""
                sq = p2.tile([128, KC, TT], BF16, tag=f"sq")
                nc.vector.tensor_mul(sq, xin_b, xin_b)
                ps_stat = ps_stat_p.tile([33, TT], F32, tag="ps")
                for k in range(KC):
                    nc.tensor.matmul(ps_stat[0:1, :], ones_k, xin_b[:, k, :],
                                     start=(k == 0), stop=(k == KC - 1))
                for k in range(KC):
                    nc.tensor.matmul(ps_stat[32:33, :], ones_k, sq[:, k, :],
                                     start=(k == 0), stop=(k == KC - 1))
                muf = pst.tile([1, TT], F32, tag="muf")
                nc.scalar.activation(muf, ps_stat[0:1, :], AF.Copy, scale=1.0 / C)
                msqf = pst.tile([1, TT], F32, tag="msqf")
                nc.scalar.activation(msqf, ps_stat[32:33, :], AF.Copy, scale=1.0 / C)
                mu2 = pst.tile([1, TT], F32, tag="mu2")
                nc.vector.tensor_mul(mu2, muf, muf)
                # msqf <- var (in-place), mu2 <- sqrt(var+eps), rf = 1/sd in msqf
                nc.vector.tensor_sub(msqf, msqf, mu2)
                nc.scalar.activation(mu2, msqf, AF.Sqrt, bias=eps_sb)
                rf = msqf
                nc.vector.reciprocal(rf, mu2)
                # per-batch broadcast rows [mu_j | r_j]
                rbm2 = pst.tile([1, 2, TT], BF16, tag="rbm2")
                MUb = p3.tile([128, TT], BF16, tag="MUb")
                Rb = p3.tile([128, TT], BF16, tag="Rb")
                for j in range(2):
                    nc.vector.tensor_copy(rbm2[0:1, j, 0:T], muf[0:1, j * T:(j + 1) * T])
                    nc.vector.tensor_copy(rbm2[0:1, j, T:TT], rf[0:1, j * T:(j + 1) * T])
                    ps_bc = ps_bc_p.tile([128, TT], F32, tag="ps")
                    nc.tensor.matmul(ps_bc, ones_b, rbm2[0:1, j, :],
                                     start=True, stop=True)
                    nc.scalar.activation(MUb[:, j * T:(j + 1) * T], ps_bc[:, 0:T], AF.Copy)
                    nc.scalar.activation(Rb[:, j * T:(j + 1) * T], ps_bc[:, T:TT], AF.Copy)
                for k in range(KC):
                    tmp = p3.tile([128, TT], BF16, tag="lntmp")
                    nc.vector.tensor_sub(tmp, xin_b[:, k, :], MUb)
                    nc.vector.tensor_mul(zout[:, k, :], tmp, Rb)

            def bail(p, src):
                nc.sync.dma_start(out=out_d[p].rearrange("(k P) t -> P k t", P=128),
                                  in_=src)

            for p in range(npair):
                xf = p2.tile([128, KC, TT], F32, tag="xf")
                nc.sync.dma_start(out=xf, in_=xf_d[p].rearrange("(k P) t -> P k t", P=128))
                xbt = p2.tile([128, KC, TT], BF16, tag="xbt")
                nc.sync.dma_start(out=xbt, in_=xb_d[p].rearrange("(k P) t -> P k t", P=128))
                if stage <= 1:
                    bail(p, xf)
                    continue

                # ---- LN1 ----
                zb = p2.tile([128, KC, TT], BF16, tag="zb")
                layernorm(xbt, xf, zb, "ln1")
                if stage <= 2:
                    zf = p3.tile([128, KC, TT], F32, tag="zf_dbg")
                    nc.vector.tensor_copy(zf, zb)
                    bail(p, zf)
                    continue

                # ---- QKV ----
                qTb = p2.tile([128, KC, TT], BF16, tag="qTb")
                kTb = p2.tile([128, KC, TT], BF16, tag="kTb")
                for (wsb, dst, bcol) in ((wq_sb, qTb, 0), (wk_sb, kTb, 3)):
                    for m in range(KC):
                        ps = ps_w_p.tile([128, TT], F32, tag="ps")
                        for k in range(KC):
                            nc.tensor.matmul(ps, wsb[:, k * C + m * 128: k * C + (m + 1) * 128],
                                             zb[:, k, :], start=(k == 0), stop=(k == KC - 1))
                        nc.scalar.activation(dst[:, m, :], ps, AF.Identity,
                                             bias=bias_sb[:, bcol + m: bcol + m + 1])
                # v (token-major, per batch j and s-tile si)
                vb = {}
                for j in range(2):
                    for si in range(2):
                        ps = ps_w_p.tile([128, C], F32, tag="ps")
                        for k in range(KC):
                            nc.tensor.matmul(
                                ps, zb[:, k, j * T + si * 128: j * T + (si + 1) * 128],
                                wv_sb[:, k * C:(k + 1) * C],
                                start=(k == 0), stop=(k == KC - 1))
                        vt = p2.tile([128, C], BF16, tag=f"vb_{j}_{si}")
                        nc.vector.tensor_add(vt, ps, bv_sb)
                        vb[(j, si)] = vt
                if stage <= 3:
                    qf = p3.tile([128, KC, TT], F32, tag="zf_dbg")
                    nc.vector.tensor_copy(qf, qTb)
                    bail(p, qf)
                    continue

                # ---- attention ----
                attnTb = p2.tile([128, KC, TT], BF16, tag="attnTb")
                for j in range(2):
                    for hp in range(KC):
                        vh0 = pe3.tile([128, 128], BF16, tag=f"vh0_{j}")
                        vh1 = pe3.tile([128, 128], BF16, tag=f"vh1_{j}")
                        Es = {}
                        for off in (0, 64):
                            h = 2 * hp + (off // 64)
                            ps_s0 = ps_s_p.tile([128, T], F32, tag="ps")
                            nc.tensor.matmul(
                                ps_s0,
                                kTb[off:off + 64, hp, j * T: j * T + 128],
                                qTb[off:off + 64, hp, j * T: (j + 1) * T],
                                start=True, stop=True, tile_position=(off, 0))
                            ps_s1 = ps_s_p.tile([128, 128], F32, tag="ps")
                            nc.tensor.matmul(
                                ps_s1,
                                kTb[off:off + 64, hp, j * T + 128: (j + 1) * T],
                                qTb[off:off + 64, hp, j * T + 128: (j + 1) * T],
                                start=True, stop=True, tile_position=(off, 0))
                            E0 = pe3.tile([128, T], BF16, tag="E0")
                            E1 = pe3.tile([128, 128], BF16, tag="E1")
                            nc.scalar.activation(E0, ps_s0, AF.Exp)
                            nc.scalar.activation(E1, ps_s1, AF.Exp)
                            nc.gpsimd.tensor_mul(E0[:, 0:128], E0[:, 0:128], mask_sb)
                            nc.gpsimd.tensor_mul(E1, E1, mask_sb)
                            S0 = pe3.tile([128, 1], F32, tag="S0")
                            S1 = pe3.tile([128, 1], F32, tag="S1")
                            nc.vector.reduce_sum(out=S0, in_=E0,
                                                 axis=mybir.AxisListType.X)
                            nc.vector.reduce_sum(out=S1, in_=E1,
                                                 axis=mybir.AxisListType.X)
                            R0 = pe3.tile([128, 1], F32, tag="R0")
                            R1 = pe3.tile([128, 1], F32, tag="R1")
                            nc.vector.reciprocal(R0, S0)
                            nc.vector.reciprocal(R1, S1)
                            nc.vector.tensor_scalar_mul(
                                vh0[:, off:off + 64],
                                vb[(j, 0)][:, h * HS:(h + 1) * HS], R0)
                            nc.vector.tensor_scalar_mul(
                                vh1[:, off:off + 64],
                                vb[(j, 1)][:, h * HS:(h + 1) * HS], R1)
                            Es[off] = (E0, E1)
                        ps_a = ps_a_p.tile([128, T], F32, tag="ps")
                        for off in (0, 64):
                            E0, E1 = Es[off]
                            nc.tensor.matmul(ps_a[off:off + 64, 0:T],
                                             vh0[:, off:off + 64], E0,
                                             start=True, stop=False,
                                             tile_position=(0, off),
                                             skip_group_check=True)
                            nc.tensor.matmul(ps_a[off:off + 64, 128:T],
                                             vh1[:, off:off + 64], E1,
                                             start=False, stop=True,
                                             tile_position=(0, off),
                                             skip_group_check=True)
                        nc.scalar.activation(attnTb[:, hp, j * T:(j + 1) * T], ps_a,
                                             AF.Copy)
                if stage <= 4:
                    af = p3.tile([128, KC, TT], F32, tag="zf_dbg")
                    nc.vector.tensor_copy(af, attnTb)
                    bail(p, af)
                    continue

                # ---- proj + residual ----
                x2f = p2.tile([128, KC, TT], F32, tag="x2f")
                for m in range(KC):
                    ps = ps_w_p.tile([128, TT], F32, tag="ps")
                    for k in range(KC):
                        nc.tensor.matmul(ps, wp_sb[:, k * C + m * 128: k * C + (m + 1) * 128],
                                         attnTb[:, k, :], start=(k == 0), stop=(k == KC - 1))
                    ytmp = p3.tile([128, TT], F32, tag="ytmp")
                    nc.scalar.activation(ytmp, ps, AF.Identity,
                                         bias=bias_sb[:, 6 + m: 7 + m])
                    nc.vector.tensor_add(x2f[:, m, :], ytmp, xf[:, m, :])
                x2b = p2.tile([128, KC, TT], BF16, tag="x2b")
                nc.gpsimd.tensor_copy(x2b, x2f)

                if stage <= 5:
                    bail(p, x2f)
                    continue

                # ---- LN2 ----
                z2b = p2.tile([128, KC, TT], BF16, tag="z2b")
                layernorm(x2b, x2f, z2b, "ln2")

                # ---- MLP ----
                ub = pu.tile([128, MU, TT], BF16, tag="ub")
                for m in range(MU):
                    ps = ps_w_p.tile([128, TT], F32, tag="ps")
                    for k in range(KC):
                        nc.tensor.matmul(
                            ps, w1_sb[:, k * 4 * C + m * 128: k * 4 * C + (m + 1) * 128],
                            z2b[:, k, :], start=(k == 0), stop=(k == KC - 1))
                    if m % 2 == 0:
                        nc.vector.tensor_scalar(
                            out=ub[:, m, :], in0=ps,
                            scalar1=bias_sb[:, 9 + m: 10 + m], scalar2=0.0,
                            op0=OP.add, op1=OP.max)
                    else:
                        nc.scalar.activation(ub[:, m, :], ps, AF.Relu,
                                             bias=bias_sb[:, 9 + m: 10 + m])
                outf = p2.tile([128, KC, TT], F32, tag="outf")
                for m in range(KC):
                    ps = ps_w_p.tile([128, TT], F32, tag="ps")
                    for k in range(MU):
                        nc.tensor.matmul(ps, w2_sb[:, k * C + m * 128: k * C + (m + 1) * 128],
                                         ub[:, k, :], start=(k == 0), stop=(k == MU - 1))
                    otmp = p3.tile([128, TT], F32, tag="otmp")
                    nc.scalar.activation(otmp, ps, AF.Identity,
                                         bias=bias_sb[:, 21 + m: 22 + m])
                    nc.vector.tensor_add(outf[:, m, :], otmp, x2f[:, m, :])
                nc.sync.dma_start(out=out_d[p].rearrange("(k P) t -> P k t", P=128),
                                  in_=outf)

    nc.compile()
    return nc


def _get_nc():
    if "nc" not in _CACHE:
        _CACHE["nc"] = _build()
    return _CACHE["nc"]


def host_prep(x, wq, wk, wv, w_proj, b_proj, w1, b1, w2, b2,
              ln1_g, ln1_b, ln2_g, ln2_b):
    f32 = np.float32
    bf16 = ml_dtypes.bfloat16
    x = np.asarray(x, f32)
    g1 = np.asarray(ln1_g, f32)
    b1n = np.asarray(ln1_b, f32)
    g2 = np.asarray(ln2_g, f32)
    b2n = np.asarray(ln2_b, f32)

    scale = f32(C) ** -0.5
    wq_all = np.asarray(wq, f32).transpose(1, 0, 2).reshape(C, C)
    wk_all = np.asarray(wk, f32).transpose(1, 0, 2).reshape(C, C)
    wv_all = np.asarray(wv, f32).transpose(1, 0, 2).reshape(C, C)
    wq2 = g1[:, None] * wq_all * scale
    wk2 = g1[:, None] * wk_all
    wv2 = g1[:, None] * wv_all
    bq = (b1n @ wq_all) * scale
    bk = b1n @ wk_all
    bv = b1n @ wv_all
    w1f = np.asarray(w1, f32)
    w1p = g2[:, None] * w1f
    b1p = np.asarray(b1, f32) + b2n @ w1f
    w2f = np.asarray(w2, f32)
    wpf = np.asarray(w_proj, f32)

    def pack(w, nk, ncols):
        return np.ascontiguousarray(
            w.reshape(nk, 128, ncols).transpose(1, 0, 2).reshape(128, nk * ncols)
        ).astype(bf16)

    wq_p = pack(wq2, KC, C)
    wk_p = pack(wk2, KC, C)
    wv_p = pack(wv2, KC, C)
    wp_p = pack(wpf, KC, C)
    w1_p = pack(w1p, KC, 4 * C)
    w2_p = pack(w2f, MU, C)

    bias_pack = np.hstack([
        bq.reshape(KC, 128).T,
        bk.reshape(KC, 128).T,
        np.asarray(b_proj, f32).reshape(KC, 128).T,
        b1p.reshape(MU, 128).T,
        np.asarray(b2, f32).reshape(KC, 128).T,
    ]).astype(f32)
    assert bias_pack.shape == (128, 24)
    trimask = np.triu(np.ones((128, 128), f32)).astype(bf16)
    bvf = bv.astype(f32)

    in_maps = []
    for c in range(NCORES):
        xc = x[c * BPC:(c + 1) * BPC]                       # [16,256,384]
        xT = np.ascontiguousarray(
            xc.reshape(NPAIR, 2, T, C).transpose(0, 3, 1, 2).reshape(NPAIR, C, TT))
        in_maps.append({
            "xf": xT,
            "xb": xT.astype(bf16),
            "wq": wq_p, "wk": wk_p, "wv": wv_p, "wp": wp_p,
            "w1": w1_p, "w2": w2_p,
            "biases": bias_pack, "bv": bvf, "trimask": trimask,
        })
    return in_maps


def kernel(**inputs):
    in_maps = host_prep(**inputs)
    nc = _get_nc()
    trace = os.environ.get("BASS_KERNEL_TRACE", "") not in ("", "0")
    res = run_bass_kernel_spmd(nc, in_maps, list(range(NCORES)), trace=trace)
    if trace and res.exec_time_ns is not None:
        print(f"HW exec time: {res.exec_time_ns} ns")
        _CACHE["exec_time_ns"] = res.exec_time_ns

    out = np.empty((B, T, C), np.float32)
    for c in range(NCORES):
        oc = res.results[c]["out"]                          # [NPAIR, C, TT]
        out[c * BPC:(c + 1) * BPC] = (
            oc.reshape(NPAIR, C, 2, T).transpose(0, 2, 3, 1).reshape(BPC, T, C))
    return out
